# revision 6
# baseline (speedup 1.0000x reference)
"""Multi-head attention (B=4, S=2048, D=1024, H=16, dk=64) on 8 TRN2 cores.

Sharding: data-parallel over B (4 batches) x tensor-parallel over head
groups (2 groups of 8 heads).  Core c handles batch c//2 and head group
c%2: it computes Q/K/V with the 512-column slice of the projection
weights, runs attention for its 8 heads, and produces a partial output
projection through the matching 512-row slice of W_o.  The host sums the
two partials per batch and adds the constant bias term (bv @ Wo^T + bo).

Per-core kernel layout notes:
  - x is fed pre-transposed (xT [D, S]) so the contraction dim d lands on
    SBUF partitions for the Q/K projections.
  - Q^T, K^T are built in [e, s] layout (e on partitions) so scores can
    be computed transposed: S^T[k, q] = (K_h^T)^T-stationary @ Q_h^T.
    Softmax runs without max subtraction (scores are O(1) here), with the
    denominator obtained by appending a ones column to V in the
    attn@V matmul, and the division applied via a rank-1 broadcast
    (ones-outer-product matmul) + elementwise multiply.
  - All matmuls run as float32r (full-rate fp32 mode, N=512 >= 256).
"""

import sys

for _p in ("/opt/trn_rl_repo",):
    if _p not in sys.path:
        sys.path.insert(0, _p)

import numpy as np
from contextlib import ExitStack

import concourse.bass as bass
import concourse.mybir as mybir
import concourse.tile as tile
from concourse import bacc
from concourse.bass_utils import run_bass_kernel_spmd

F32 = mybir.dt.float32
F32R = mybir.dt.float32r
AF = mybir.ActivationFunctionType

D, S = 1024, 2048   # d_model, seq len
E = 512             # per-core projection width (8 heads x 64)
H, DK = 8, 64       # heads per core, head dim
NB = D // 128       # contraction chunks (8)
SCALE = 0.125       # 1/sqrt(dk)


def build_bass(n_attn_et=4, do_yproj=True):
    nc = bacc.Bacc(
        "TRN2", target_bir_lowering=False, debug=False, num_devices=8
    )
    xT = nc.dram_tensor("xT", [D, S], F32R, kind="ExternalInput").ap()
    xTf = nc.dram_tensor("xTf", [D, S], F32, kind="ExternalInput").ap()
    wq = nc.dram_tensor("wq", [D, E], F32, kind="ExternalInput").ap()
    wk = nc.dram_tensor("wk", [D, E], F32, kind="ExternalInput").ap()
    wv = nc.dram_tensor("wv", [D, E], F32R, kind="ExternalInput").ap()
    wo = nc.dram_tensor("wo", [E, D], F32R, kind="ExternalInput").ap()
    bq = nc.dram_tensor("bq", [E], F32, kind="ExternalInput").ap()
    bk = nc.dram_tensor("bk", [E], F32, kind="ExternalInput").ap()
    y = nc.dram_tensor("y", [S, D], F32, kind="ExternalOutput").ap()

    with ExitStack() as ctx:
        tc = ctx.enter_context(tile.TileContext(nc))
        const = ctx.enter_context(tc.tile_pool(name="const", bufs=1))
        wpool = ctx.enter_context(tc.tile_pool(name="wpool", bufs=8))
        xpool = ctx.enter_context(tc.tile_pool(name="xpool", bufs=10))
        qkpool = ctx.enter_context(tc.tile_pool(name="qkpool", bufs=2))
        res = ctx.enter_context(tc.tile_pool(name="res", bufs=1))
        epool = ctx.enter_context(tc.tile_pool(name="epool", bufs=3))
        ypool = ctx.enter_context(tc.tile_pool(name="ypool", bufs=2))
        bcpool = ctx.enter_context(tc.tile_pool(name="bcpool", bufs=2))
        rpool = ctx.enter_context(tc.tile_pool(name="rpool", bufs=2))
        ps_s = ctx.enter_context(tc.tile_pool(name="ps_s", bufs=3, space="PSUM"))
        ps_o = ctx.enter_context(tc.tile_pool(name="ps_o", bufs=4, space="PSUM"))
        ps_b = ctx.enter_context(tc.tile_pool(name="ps_b", bufs=1, space="PSUM"))

        # ---- constants ----
        bq_t = const.tile([128, 4], F32, tag="bq", name="bq_t")
        bk_t = const.tile([128, 4], F32, tag="bk", name="bk_t")
        ones_f = const.tile([128, 64], F32, tag="ones_f", name="ones_f")
        ones = const.tile([1, 64], F32R, tag="ones", name="ones_t")
        nc.sync.dma_start(bq_t[:, :], bq.rearrange("(j p) -> p j", p=128))
        nc.sync.dma_start(bk_t[:, :], bk.rearrange("(j p) -> p j", p=128))
        nc.vector.memset(ones_f[:, :], 1.0)
        nc.scalar.copy(ones[:, :], ones_f[0:1, :])

        # ---- residents: V (with interleaved ones cols) and attn-out^T ----
        vt = [
            res.tile([128, H, 65], F32R, tag="vt", bufs=16, name=f"vt{i}")
            for i in range(16)
        ]
        ao = [
            res.tile([128, S], F32R, tag="ao", bufs=4, name=f"ao{i}")
            for i in range(4)
        ]

        def load_x_stripe(sc, label, dt_, src_):
            xs = []
            for dc in range(NB):
                xt_ = xpool.tile(
                    [128, 512], dt_, tag="xs", name=f"x_{label}_{sc}_{dc}"
                )
                nc.sync.dma_start(
                    xt_[:, :],
                    src_[dc * 128 : (dc + 1) * 128, sc * 512 : (sc + 1) * 512],
                )
                xs.append(xt_)
            return xs

        # ---- V projection (x stationary, Wv moving) ----
        wv_t = []
        for dc in range(NB):
            wvt = wpool.tile([128, 512], F32R, tag="w", name=f"wv{dc}")
            nc.sync.dma_start(wvt[:, :], wv[dc * 128 : (dc + 1) * 128, :])
            wv_t.append(wvt)
        for sc in range(4):
            xs = load_x_stripe(sc, "v", F32R, xT)
            for st in range(4):
                s_abs = sc * 4 + st
                vp = ps_s.tile([128, 512], F32, tag="s", name=f"vp{s_abs}")
                for dc in range(NB):
                    nc.tensor.matmul(
                        vp[:, :],
                        (xs[dc][:, st * 128 : (st + 1) * 128]),
                        (wv_t[dc][:, :]),
                        start=(dc == 0),
                        stop=(dc == NB - 1),
                    )
                nc.scalar.copy(
                    vt[s_abs][:, :, 0:64], vp.rearrange("p (h d) -> p h d", h=H)
                )
                nc.scalar.copy(
                    vt[s_abs][:, :, 64:65],
                    ones_f[:, 0:8].rearrange("p (h o) -> p h o", o=1),
                )

        # ---- per head-group-of-2 (one e-tile): Q/K projection + attention ----
        for et in range(4):
            wq_t = wpool.tile([128, NB, 128], F32, tag="w", name=f"wq{et}")
            nc.sync.dma_start(
                wq_t[:, :, :],
                wq.rearrange("(dc p) e -> p dc e", p=128)[
                    :, :, et * 128 : (et + 1) * 128
                ],
            )
            wk_t = wpool.tile([128, NB, 128], F32, tag="w", name=f"wk{et}")
            nc.sync.dma_start(
                wk_t[:, :, :],
                wk.rearrange("(dc p) e -> p dc e", p=128)[
                    :, :, et * 128 : (et + 1) * 128
                ],
            )
            qT_t = qkpool.tile([128, S], F32R, tag="qT", name=f"qT{et}")
            kT_t = qkpool.tile([128, S], F32R, tag="kT", name=f"kT{et}")
            for sc in range(4):
                xs = load_x_stripe(sc, f"qk{et}", F32, xTf)
                for wt_, dst, bias_t in ((wq_t, qT_t, bq_t), (wk_t, kT_t, bk_t)):
                    pp = ps_s.tile([128, 512], F32, tag="s", name=f"pp{et}_{sc}")
                    for dc in range(NB):
                        nc.tensor.matmul(
                            pp[:, :],
                            (wt_[:, dc, :]),
                            (xs[dc][:, :]),
                            start=(dc == 0),
                            stop=(dc == NB - 1),
                        )
                    nc.scalar.add(
                        dst[:, sc * 512 : (sc + 1) * 512],
                        pp[:, :],
                        bias_t[:, et : et + 1],
                    )

            for hh in range(2 if et < n_attn_et else 0):
                h = 2 * et + hh
                off = hh * 64
                o_ps = [
                    ps_o.tile([65, 512], F32, tag="o", name=f"o{h}_{qc}")
                    for qc in range(4)
                ]
                for kt in range(16):
                    eps = []
                    for qc in range(4):
                        sp = ps_s.tile([128, 512], F32, tag="s", name=f"sp{h}_{kt}_{qc}")
                        nc.tensor.matmul(
                            sp[:, :],
                            (kT_t[off : off + 64, kt * 128 : (kt + 1) * 128]),
                            (qT_t[off : off + 64, qc * 512 : (qc + 1) * 512]),
                            start=True,
                            stop=True,
                        )
                        ep = epool.tile([128, 512], F32R, tag="e", name=f"ep{h}_{kt}_{qc}")
                        nc.scalar.activation(ep[:, :], sp[:, :], AF.Exp, scale=SCALE)
                        eps.append(ep)
                    for qc in range(4):
                        nc.tensor.matmul(
                            o_ps[qc][:, :],
                            (vt[kt][:, h, :]),
                            (eps[qc][:, :]),
                            start=(kt == 0),
                            stop=(kt == 15),
                        )
                for qc in range(4):
                    recip = rpool.tile([1, 512], F32R, tag="r", name=f"rc{h}_{qc}")
                    with nc.allow_low_precision("f32r recip of softmax denom"):
                        nc.vector.reciprocal(recip[:, :], o_ps[qc][64:65, :])
                    bc_ps = ps_b.tile([64, 512], F32, tag="b", name=f"bp{h}_{qc}")
                    nc.tensor.matmul(
                        bc_ps[:, :], (ones[:, :]), (recip[:, :]),
                        start=True, stop=True,
                    )
                    bc_sb = bcpool.tile([64, 512], F32, tag="bc", name=f"bs{h}_{qc}")
                    nc.scalar.copy(bc_sb[:, :], bc_ps[:, :])
                    nc.vector.tensor_mul(
                        ao[et][off : off + 64, qc * 512 : (qc + 1) * 512],
                        o_ps[qc][0:64, :],
                        bc_sb[:, :],
                    )

        # ---- output projection (partial: this core's 512 e-rows of Wo) ----
        wo_t = []
        for ec in range(4):
            wot = wpool.tile([128, 1024], F32R, tag="w", name=f"wo{ec}")
            nc.sync.dma_start(wot[:, :], wo[ec * 128 : (ec + 1) * 128, :])
            wo_t.append(wot)
        for qt in range(16 if do_yproj else 0):
            yps = [
                ps_s.tile([128, 512], F32, tag="s", name=f"yp{qt}_{oc}")
                for oc in range(2)
            ]
            for ec in range(4):
                for oc in range(2):
                    nc.tensor.matmul(
                        yps[oc][:, :],
                        (ao[ec][:, qt * 128 : (qt + 1) * 128]),
                        (wo_t[ec][:, oc * 512 : (oc + 1) * 512]),
                        start=(ec == 0),
                        stop=(ec == 3),
                    )
            ysb = ypool.tile([128, 1024], F32, tag="y", name=f"ysb{qt}")
            for oc in range(2):
                nc.scalar.copy(ysb[:, oc * 512 : (oc + 1) * 512], yps[oc][:, :])
            nc.sync.dma_start(y[qt * 128 : (qt + 1) * 128, :], ysb[:, :])

    nc.finalize()
    return nc


def make_in_maps(x, Wq, Wk, Wv, Wo, bq, bk):
    in_maps = []
    for c in range(8):
        b, g = divmod(c, 2)
        sl = slice(g * E, (g + 1) * E)
        in_maps.append(
            {
                "xT": np.ascontiguousarray(x[b].T, dtype=np.float32),
                "xTf": np.ascontiguousarray(x[b].T, dtype=np.float32),
                "wq": np.ascontiguousarray(Wq[sl, :].T, dtype=np.float32),
                "wk": np.ascontiguousarray(Wk[sl, :].T, dtype=np.float32),
                "wv": np.ascontiguousarray(Wv[sl, :].T, dtype=np.float32),
                "wo": np.ascontiguousarray(Wo[:, sl].T, dtype=np.float32),
                "bq": np.ascontiguousarray(bq[sl], dtype=np.float32),
                "bk": np.ascontiguousarray(bk[sl], dtype=np.float32),
            }
        )
    return in_maps


_NC = None


def run(x, Wq, bq, Wk, bk, Wv, bv, Wo, bo, build_kwargs=None, **run_kwargs):
    global _NC
    x = np.asarray(x, dtype=np.float32)
    Wq, Wk, Wv, Wo = (np.asarray(a, dtype=np.float32) for a in (Wq, Wk, Wv, Wo))
    bq, bk, bv, bo = (np.asarray(a, dtype=np.float32) for a in (bq, bk, bv, bo))
    if _NC is None:
        _NC = build_bass(**(build_kwargs or {}))
    res = run_bass_kernel_spmd(
        _NC, make_in_maps(x, Wq, Wk, Wv, Wo, bq, bk),
        core_ids=list(range(8)), **run_kwargs,
    )
    ys = [r["y"] for r in res.results]
    c_vec = (bv @ Wo.T + bo).astype(np.float32)  # constant bias fold
    out = np.stack([ys[2 * b] + ys[2 * b + 1] + c_vec for b in range(4)])
    return out.astype(np.float32), res


def kernel(x, Wq, bq, Wk, bk, Wv, bv, Wo, bo):
    out, _ = run(x, Wq, bq, Wk, bk, Wv, bv, Wo, bo)
    return out
